# revision 33
# baseline (speedup 1.0000x reference)
"""Trainium2 Bass kernel for nn_MoE_Disentangled (moe_routing).

Contract: kernel(**inputs) takes FULL unsharded inputs (as from
reference.setup_inputs()) and returns the FULL outputs
(expert_features [4,16,1024], confidence [4,16,1], fused [4,1024]).

Math: only the first E=16 tokens of the sequence reach the outputs.
Attention mixes tokens (so k/v are needed over all S=1040 tokens), but
only the E expert-token queries matter; everything after attention is
row-wise, so the MoE runs on just B*E=64 tokens.  This is exact.

Sharding over 8 NeuronCores:
  - attention: head-parallel (core c owns head c); k_h/v_h projections over
    all tokens need no communication; o@Wproj row-block partials are
    AllReduce-summed (256 KB).
  - MoE: expert-parallel (core c owns experts 2c,2c+1); weighted
    accumulator partials AllReduce-summed (256 KB).
Attention/routing path is fp32 end-to-end (top-2 routing is sensitive to
score perturbations); expert MLP weights stream as bf16 with fp32 PSUM
accumulation.
"""

import numpy as np
import ml_dtypes

import concourse.bass as bass
import concourse.bacc as bacc
import concourse.tile as tile
import concourse.mybir as mybir
from concourse.bass_utils import run_bass_kernel_spmd
from concourse.masks import make_identity
from concourse.bass import _add_dep_helper

F32 = mybir.dt.float32
BF16 = mybir.dt.bfloat16
F32R = mybir.dt.float32r
AF = mybir.ActivationFunctionType
OP = mybir.AluOpType

B, N, D = 4, 1024, 1024
E, NH = 16, 8
H = 2 * D
HD = D // NH
EPS = 1e-5
NCORES = 8
EPC = E // NCORES          # experts per core
NT = 33                    # token tiles: tile 0 = expert tokens (16 rows), 1..32 = inputs
BE = B * E                 # 64 routed tokens
SCALE = HD ** -0.5


def _bcast_ap(handle, parts, free):
    """AP reading a [1, free] DRAM row broadcast to `parts` partitions."""
    return bass.AP(handle.ap().tensor, 0, [[0, parts], [1, free]])


def build_program():
    nc = bacc.Bacc("TRN2", target_bir_lowering=False, debug=False,
                   num_devices=NCORES)
    def din(name, shape, dtype=F32):
        return nc.dram_tensor(name, list(shape), dtype, kind="ExternalInput")

    x_in = din("x_in", [B * N, D])               # inputs reshaped
    xtop = din("xtop", [E, D])                   # expert_tokens_top
    xtb = din("xtb", [E, D])                     # expert_tokens_top + bproj
    wk = din("wk", [8, HD, HD])                  # ln1_s-folded Wkv k-slice, [d][128,128]
    wv = din("wv", [8, HD, HD])
    wq = din("wq", [8, HD, HD])
    bk = din("bk", [HD, 1])                      # (ln1_b @ Wkv) k-slice
    bq = din("bq", [HD, 1])
    bv = din("bv", [HD, 1])
    wproj = din("wproj", [HD, D])                # Wproj rows of this head
    s2row = din("s2row", [1, D])                 # ln2_s
    b2row = din("b2row", [1, D])                 # ln2_b
    moew = din("moew", [8, HD, E])               # moe_expert_tokens.T chunks
    esel = din("esel", [E, EPC])                 # one-hot picks of this core's experts
    w1 = din("w1", [EPC, 8, HD, H], BF16)
    b1r = din("b1r", [EPC, H], BF16)
    w2 = din("w2", [EPC, 16, HD, H], BF16)
    b2r = din("b2r", [EPC, H], BF16)
    w3 = din("w3", [EPC, 16, HD, D], BF16)
    b3r = din("b3r", [EPC, D], BF16)
    rsel = din("rsel", [BE, 8])                  # one-hot rows of this core's shard
    wgrow = din("wgrow", [1, D])
    bgrow = din("bgrow", [1, 1])

    ef_out = nc.dram_tensor("ef", [8, D], F32, kind="ExternalOutput")
    conf_out = nc.dram_tensor("conf", [8, 1], F32, kind="ExternalOutput")
    fused_out = nc.dram_tensor("fused", [B, D], F32, kind="ExternalOutput")

    with tile.TileContext(nc) as tc:
        with (
            tc.tile_pool(name="consts", bufs=1) as consts,
            tc.tile_pool(name="persist", bufs=1) as persist,
            tc.tile_pool(name="work", bufs=1) as work,
            tc.tile_pool(name="small", bufs=8) as small,
            tc.tile_pool(name="dram", bufs=1, space="DRAM") as dram,
        ):
            # ---- constants ----
            ident = consts.tile([128, 128], F32)
            make_identity(nc, ident)
            ident_bf = consts.tile([64, 64], BF16)
            make_identity(nc, ident_bf)
            ones_row = consts.tile([1, 128], F32)
            nc.vector.memset(ones_row, 1.0)
            ones_row_bf = consts.tile([1, BE], BF16)
            nc.vector.memset(ones_row_bf, 1.0)
            eps_t = consts.tile([128, 1], F32)
            nc.vector.memset(eps_t, EPS)

            wk_st = work.tile([HD, 8, HD], F32, tag="wstage")
            nc.sync.dma_start(out=wk_st, in_=wk.ap().rearrange("d r c -> r d c"))
            wk_sb = consts.tile([HD, 8, HD], F32R)
            nc.vector.tensor_copy(wk_sb, wk_st)
            wv_st = work.tile([HD, 8, HD], F32, tag="wstage")
            nc.sync.dma_start(out=wv_st, in_=wv.ap().rearrange("d r c -> r d c"))
            wv_sb = consts.tile([HD, 8, HD], F32R)
            nc.vector.tensor_copy(wv_sb, wv_st)
            wq_st = work.tile([HD, 8, HD], F32, tag="wstage")
            nc.sync.dma_start(out=wq_st, in_=wq.ap().rearrange("d r c -> r d c"))
            wq_sb = consts.tile([HD, 8, HD], F32R)
            nc.vector.tensor_copy(wq_sb, wq_st)
            moew_sb = consts.tile([HD, 8, E], F32)
            nc.sync.dma_start(out=moew_sb, in_=moew.ap().rearrange("d r c -> r d c"))
            wp_st = work.tile([HD, D], F32, tag="wstage")
            nc.sync.dma_start(out=wp_st, in_=wproj.ap())
            wproj_sb = consts.tile([HD, D], F32R)
            nc.vector.tensor_copy(wproj_sb, wp_st)
            bk_sb = consts.tile([HD, 1], F32)
            nc.sync.dma_start(out=bk_sb, in_=bk.ap())
            bq_sb = consts.tile([HD, 1], F32)
            nc.sync.dma_start(out=bq_sb, in_=bq.ap())
            bv_sb = consts.tile([HD, 1], F32)
            nc.sync.dma_start(out=bv_sb, in_=bv.ap())
            esel_sb = consts.tile([E, EPC], F32)
            nc.sync.dma_start(out=esel_sb, in_=esel.ap())
            rsel_sb = consts.tile([BE, 8], F32)
            nc.sync.dma_start(out=rsel_sb, in_=rsel.ap())

            xtb_sb = consts.tile([BE, D], F32)   # x_top + bproj, broadcast to 64 rows
            nc.sync.dma_start(
                out=xtb_sb, in_=bass.AP(xtb.ap().tensor, 0, [[0, B], [D, E], [1, D]]))
            s2b_sb = consts.tile([BE, D], F32)
            nc.sync.dma_start(out=s2b_sb, in_=_bcast_ap(s2row, BE, D))
            b2b_sb = consts.tile([BE, D], F32)
            nc.sync.dma_start(out=b2b_sb, in_=_bcast_ap(b2row, BE, D))
            wgb_sb = consts.tile([BE, D], F32)
            nc.sync.dma_start(out=wgb_sb, in_=_bcast_ap(wgrow, BE, D))
            bgb_sb = consts.tile([BE, 1], F32)
            nc.sync.dma_start(out=bgb_sb, in_=_bcast_ap(bgrow, BE, 1))

            # ---- persistent results ----
            x2_sb = persist.tile([BE, D], F32)           # post-attention residual

            x_in_r = x_in.ap().rearrange("(b n) d -> b n d", n=N)

            # MoE weight stream pool opened early: lets the scheduler prefetch
            # expert weights during phase 1 and the AllReduce gap.
            wstream = ctx_wstream = tc.tile_pool(name="wstream", bufs=10)
            wstream = wstream.__enter__()

            # ===== phase 1: LN1 + per-head projections over all tokens =====
            kvpool = tc.tile_pool(name="kvpool", bufs=1)
            kvp = kvpool.__enter__()
            k_all = kvp.tile([HD, NT, 128], F32)      # k feature-major per tile
            v_all = kvp.tile([128, NT, HD + 1], F32)  # v token-major (+ones col)
            qT_sb = kvp.tile([HD, E], F32)            # q^T for the 16 expert tokens
            G = 2                                     # input tiles per group
            with (
                tc.tile_pool(name="xp", bufs=4) as xp,
                tc.tile_pool(name="xnp", bufs=3) as xnp,
                tc.tile_pool(name="xntp", bufs=3) as xntp,
                tc.tile_pool(name="vfm", bufs=2) as vfmp,
                tc.tile_pool(name="pst", bufs=4, space="PSUM") as pst,
                tc.tile_pool(name="pskv", bufs=4, space="PSUM") as pskv,
            ):
                nonlocal_sqrt = [None]

                def ln_tile(x_t, P_t, xnT, col):
                    """LN stats + normalize + 8 transposes into xnT[:, :, col:col+P_t]."""
                    stats = small.tile([128, 2, 6], F32, tag="stats")
                    nc.vector.bn_stats(stats[0:P_t, 0, :], x_t[0:P_t, 0:512])
                    nc.vector.bn_stats(stats[0:P_t, 1, :], x_t[0:P_t, 512:1024])
                    mv = small.tile([128, 2], F32, tag="mv")
                    nc.vector.bn_aggr(mv[0:P_t, :], stats[0:P_t, :, :])
                    r_t = small.tile([128, 1], F32, tag="r")
                    nonlocal_sqrt[0] = nc.scalar.activation(
                        r_t[0:P_t, :], mv[0:P_t, 1:2], AF.Sqrt,
                        bias=eps_t[0:P_t, :])
                    nc.vector.reciprocal(r_t[0:P_t, :], r_t[0:P_t, :])
                    nmr = small.tile([128, 1], F32, tag="nmr")
                    nc.vector.tensor_scalar(nmr[0:P_t, :], mv[0:P_t, 0:1],
                                            r_t[0:P_t, :], -1.0,
                                            op0=OP.mult, op1=OP.mult)
                    xn = xnp.tile([128, D], F32, tag="xn")
                    nc.scalar.activation(xn[0:P_t, 0:512], x_t[0:P_t, 0:512],
                                         AF.Identity,
                                         bias=nmr[0:P_t, :], scale=r_t[0:P_t, :])
                    nc.vector.tensor_scalar(xn[0:P_t, 512:1024], x_t[0:P_t, 512:1024],
                                            r_t[0:P_t, :], nmr[0:P_t, :],
                                            op0=OP.mult, op1=OP.add)
                    for dq in range(2):
                        tp = pst.tile([128, 4, 128], F32, tag="tp")
                        for q in range(4):
                            d = dq * 4 + q
                            nc.tensor.transpose(tp[:, q, 0:P_t],
                                                xn[0:P_t, d * 128:(d + 1) * 128],
                                                ident[0:P_t, 0:P_t])
                        if dq == 0:
                            nc.scalar.copy(xnT[:, 0:4, col:col + P_t],
                                           tp[:, :, 0:P_t])
                        else:
                            nc.vector.tensor_copy(xnT[:, 4:8, col:col + P_t],
                                                  tp[:, :, 0:P_t])

                # --- expert-token tile (16 rows, t=0) ---
                x_t = xp.tile([128, D], F32, tag="x_t")
                nc.sync.dma_start(out=x_t[0:E, 0:512], in_=xtop.ap()[:, 0:512])
                nc.sync.dma_start(out=x_t[0:E, 512:1024], in_=xtop.ap()[:, 512:1024])
                xnT0 = xntp.tile([HD, 8, G * 128], F32R, tag="xnT")
                ln_tile(x_t, E, xnT0, 0)
                kp = pskv.tile([128, G * 128], F32, tag="kvq")
                for d in range(8):
                    nc.tensor.matmul(kp[:, 0:E], wk_sb[:, d, :], xnT0[:, d, 0:E],
                                     start=(d == 0), stop=(d == 7))
                nc.scalar.activation(k_all[:, 0, 0:E], kp[:, 0:E], AF.Identity,
                                     bias=bk_sb)
                vp = pskv.tile([128, G * 128], F32, tag="kvq")
                for d in range(8):
                    nc.tensor.matmul(vp[:, 0:E], wv_sb[:, d, :], xnT0[:, d, 0:E],
                                     start=(d == 0), stop=(d == 7))
                vfm = vfmp.tile([HD, G * 128], F32, tag="vfm")
                nc.scalar.activation(vfm[:, 0:E], vp[:, 0:E], AF.Identity,
                                     bias=bv_sb)
                tp = pst.tile([128, 4, 128], F32, tag="tp")
                nc.tensor.transpose(tp[0:E, 0, :], vfm[:, 0:E], ident)
                nc.vector.tensor_copy(v_all[0:E, 0, 0:HD], tp[0:E, 0, :])
                nc.vector.memset(v_all[0:E, 0, HD:HD + 1], 1.0)
                qp = pskv.tile([128, G * 128], F32, tag="kvq")
                for d in range(8):
                    nc.tensor.matmul(qp[:, 0:E], wq_sb[:, d, :], xnT0[:, d, 0:E],
                                     start=(d == 0), stop=(d == 7))
                nc.scalar.activation(qT_sb[:, :], qp[:, 0:E], AF.Identity,
                                     bias=bq_sb)

                # --- input tiles, G at a time (grouped stationary loads) ---
                for g in range(32 // G):
                    t0 = 1 + g * G
                    xnT = xntp.tile([HD, 8, G * 128], F32R, tag="xnT")
                    for i in range(G):
                        x_t = xp.tile([128, D], F32, tag="x_t")
                        rows = x_in.ap()[(t0 - 1 + i) * 128:(t0 + i) * 128, :]
                        nc.sync.dma_start(out=x_t[:, 0:512], in_=rows[:, 0:512])
                        nc.sync.dma_start(out=x_t[:, 512:1024], in_=rows[:, 512:1024])
                        ln_tile(x_t, 128, xnT, i * 128)
                    # k for the whole group (feature-major, contiguous tiles)
                    kp = pskv.tile([128, G * 128], F32, tag="kvq")
                    for d in range(8):
                        nc.tensor.matmul(kp[:, :], wk_sb[:, d, :], xnT[:, d, :],
                                         start=(d == 0), stop=(d == 7))
                    nc.scalar.activation(k_all[:, t0:t0 + G, :], kp[:, :],
                                         AF.Identity, bias=bk_sb)
                    # v feature-major for the group, then transpose per tile
                    vp = pskv.tile([128, G * 128], F32, tag="kvq")
                    for d in range(8):
                        nc.tensor.matmul(vp[:, :], wv_sb[:, d, :], xnT[:, d, :],
                                         start=(d == 0), stop=(d == 7))
                    vfm = vfmp.tile([HD, G * 128], F32, tag="vfm")
                    nc.scalar.activation(vfm[:, :], vp[:, :], AF.Identity,
                                         bias=bv_sb)
                    tp = pst.tile([128, 4, 128], F32, tag="tp")
                    for i in range(G):
                        nc.tensor.transpose(tp[:, i, :], vfm[:, i * 128:(i + 1) * 128],
                                            ident)
                    nc.vector.tensor_copy(v_all[:, t0:t0 + G, 0:HD], tp[:, 0:G, :])
                    nc.vector.memset(v_all[:, t0:t0 + G, HD:HD + 1], 1.0)

            # ===== attention for this head (16 queries per batch) =====
            last_sqrt = nonlocal_sqrt[0]
            ar1_in = dram.tile([BE, D], F32)
            ar1_out = dram.tile([BE, D], F32)
            with (
                tc.tile_pool(name="attn", bufs=4) as attn,
                tc.tile_pool(name="pst", bufs=2, space="PSUM") as pst,
                tc.tile_pool(name="psa", bufs=2, space="PSUM") as psa,
                tc.tile_pool(name="psop", bufs=1, space="PSUM") as psop,
            ):
                for b in range(B):
                    tiles_b = [0] + list(range(8 * b + 1, 8 * b + 9))
                    po = psa.tile([E, HD + 1], F32, tag="po")
                    for i, t in enumerate(tiles_b):
                        P_t = E if t == 0 else 128
                        ps_s = psa.tile([128, E], F32, tag="ps_s")
                        nc.tensor.matmul(ps_s[0:P_t, :], k_all[:, t, 0:P_t], qT_sb,
                                         start=True, stop=True)
                        et = attn.tile([128, E], F32, tag="et")
                        exp_i = nc.scalar.activation(et[0:P_t, :], ps_s[0:P_t, :],
                                                     AF.Exp, scale=SCALE)
                        _add_dep_helper(exp_i.ins, last_sqrt.ins, sync=True,
                                        reason="ACT table: exp after all LN1 sqrt")
                        nc.tensor.matmul(po[:, :], et[0:P_t, :], v_all[0:P_t, t, :],
                                         start=(i == 0), stop=(i == 8),
                                         skip_group_check=True)
                    rden = attn.tile([E, 1], F32, tag="rden")
                    nc.vector.reciprocal(rden, po[:, HD:HD + 1])
                    o_sb = attn.tile([E, HD], F32, tag="o_sb")
                    nc.scalar.activation(o_sb, po[:, 0:HD], AF.Copy, scale=rden)
                    otp = pst.tile([128, 128], F32, tag="tp")
                    nc.tensor.transpose(otp[:, 0:E], o_sb, ident[0:E, 0:E])
                    oT = attn.tile([HD, E], F32R, tag="oT")
                    nc.vector.tensor_copy(oT, otp[:, 0:E])
                    pop = psop.tile([E, D], F32, tag="pop")
                    nc.tensor.matmul(pop[:, 0:512], oT, wproj_sb[:, 0:512],
                                     start=True, stop=True)
                    nc.tensor.matmul(pop[:, 512:1024], oT, wproj_sb[:, 512:1024],
                                     start=True, stop=True)
                    attnp_b = attn.tile([E, D], F32, tag="attnp_b")
                    nc.scalar.copy(attnp_b, pop[:, :])
                    nc.sync.dma_start(out=ar1_in[E * b:E * (b + 1), :], in_=attnp_b)

            # ===== fused passthrough (fills the AllReduce gap) =====
            f0 = work.tile([B, D], F32, tag="f0")
            nc.sync.dma_start(out=f0, in_=x_in_r[:, 0, :])
            f1 = work.tile([B, D], F32, tag="attns")
            nc.sync.dma_start(out=f1, in_=x_in_r[:, 1, :])
            nc.vector.tensor_add(f0, f0, f1)
            nc.vector.tensor_scalar_mul(f0, f0, 0.5)
            nc.sync.dma_start(out=fused_out.ap(), in_=f0)

            # ===== AllReduce #1: sum head partials =====
            kvpool.__exit__(None, None, None)
            nc.gpsimd.collective_compute(
                "AllReduce", OP.add, replica_groups=[list(range(NCORES))],
                ins=[ar1_in.opt()], outs=[ar1_out.opt()])
            attns = work.tile([BE, D], F32, tag="attns")
            nc.sync.dma_start(out=attns, in_=ar1_out)
            nc.vector.tensor_add(x2_sb, attns, xtb_sb)

            # ===== LN2 + routing =====
            with (
                tc.tile_pool(name="moe", bufs=1) as moe,
            ):
                pst1_ctx = tc.tile_pool(name="pst", bufs=2, space="PSUM")
                pst = pst1_ctx.__enter__()
                psr_ctx = tc.tile_pool(name="psr", bufs=2, space="PSUM")
                psr = psr_ctx.__enter__()
                stats2 = small.tile([BE, 2, 6], F32, tag="stats")
                nc.vector.bn_stats(stats2[0:BE, 0, :], x2_sb[:, 0:512])
                nc.vector.bn_stats(stats2[0:BE, 1, :], x2_sb[:, 512:1024])
                mv2 = small.tile([BE, 2], F32, tag="mv")
                nc.vector.bn_aggr(mv2[0:BE], stats2[0:BE])
                r2 = small.tile([BE, 1], F32, tag="r")
                nc.scalar.activation(r2[0:BE], mv2[0:BE, 1:2], AF.Sqrt,
                                     bias=eps_t[0:BE, :])
                nc.vector.reciprocal(r2[0:BE], r2[0:BE])
                nmr2 = small.tile([BE, 1], F32, tag="nmr")
                nc.vector.tensor_scalar(nmr2[0:BE], mv2[0:BE, 0:1], r2[0:BE], -1.0,
                                        op0=OP.mult, op1=OP.mult)
                xn2 = work.tile([BE, D], F32, tag="xn2")
                nc.scalar.activation(xn2, x2_sb, AF.Identity, bias=nmr2[0:BE],
                                     scale=r2[0:BE])
                nc.vector.scalar_tensor_tensor(xn2, xn2, 1.0, s2b_sb,
                                               op0=OP.mult, op1=OP.mult)
                nc.vector.tensor_add(xn2, xn2, b2b_sb)

                xn2T = work.tile([HD, 8, BE], F32, tag="xn2T")
                xn2T_bf = work.tile([HD, 8, BE], BF16, tag="xn2T_bf")
                for dq in range(2):
                    tp = pst.tile([128, 4, BE], F32, tag="tp")
                    for q in range(4):
                        d = dq * 4 + q
                        nc.tensor.transpose(tp[:, q, :],
                                            xn2[:, d * 128:(d + 1) * 128],
                                            ident[0:BE, 0:BE])
                    nc.scalar.copy(xn2T[:, dq * 4:(dq + 1) * 4, :], tp[:, :, :])
                    nc.vector.tensor_copy(xn2T_bf[:, dq * 4:(dq + 1) * 4, :],
                                          tp[:, :, :])

                ps_sc = psr.tile([BE, 128], F32, tag="psr")
                for d in range(8):
                    nc.tensor.matmul(ps_sc[:, 0:E], xn2T[:, d, :], moew_sb[:, d, :],
                                     start=(d == 0), stop=(d == 7))
                sc_sb = small.tile([BE, E], F32, tag="sc_sb")
                nc.vector.tensor_copy(sc_sb, ps_sc[:, 0:E])
                m8 = small.tile([BE, 8], F32, tag="m8")
                nc.vector.max(m8, sc_sb)
                eq1 = small.tile([BE, E], F32, tag="eq1")
                nc.vector.tensor_single_scalar(eq1, sc_sb, m8[:, 0:1], op=OP.is_equal)
                eq2 = small.tile([BE, E], F32, tag="eq2")
                nc.vector.tensor_single_scalar(eq2, sc_sb, m8[:, 1:2], op=OP.is_equal)
                w_sb = small.tile([BE, E], F32, tag="w_sb")
                nc.vector.tensor_add(w_sb, eq1, eq2)
                nc.vector.tensor_scalar_mul(w_sb, w_sb, 0.5)
                # transpose w, select this core's expert columns
                wtp = pst.tile([128, 128], F32, tag="tp")
                nc.tensor.transpose(wtp[0:E, 0:BE], w_sb, ident[0:BE, 0:BE])
                wT = small.tile([E, BE], F32, tag="wT")
                nc.vector.tensor_copy(wT, wtp[0:E, 0:BE])
                wlp = psr.tile([BE, 128], F32, tag="psr")
                nc.tensor.matmul(wlp[:, 0:EPC], wT, esel_sb, start=True, stop=True)
                wloc = small.tile([BE, EPC], F32, tag="wloc")
                nc.vector.tensor_copy(wloc, wlp[:, 0:EPC])
                wrow_bf = small.tile([1, EPC, BE], BF16, tag="wrow_bf")
                for e in range(EPC):
                    wrp = psr.tile([BE, 128], F32, tag="psr")
                    nc.tensor.matmul(wrp[0:1, 0:BE], esel_sb[:, e:e + 1], wT,
                                     start=True, stop=True)
                    nc.vector.tensor_copy(wrow_bf[0:1, e, :], wrp[0:1, 0:BE])

                psr_ctx.__exit__(None, None, None)
                pst1_ctx.__exit__(None, None, None)
                # ===== expert MLPs (bf16), weighted-accumulated in PSUM =====
                psh_ctx = tc.tile_pool(name="psh", bufs=1, space="PSUM")
                psh = psh_ctx.__enter__()
                psacc_ctx = tc.tile_pool(name="psacc", bufs=1, space="PSUM")
                psacc = psacc_ctx.__enter__()
                pstq_ctx = tc.tile_pool(name="pstq", bufs=2, space="PSUM")
                pstq = pstq_ctx.__enter__()
                acc_ps = psacc.tile([BE, D], F32)
                for e in range(EPC):
                    h1g = moe.tile([BE, H], BF16, tag="h1g")
                    ph = psh.tile([BE, H], F32, tag="ph")
                    for d in range(8):
                        w1t = wstream.tile([HD, H], BF16, tag="wt")
                        nc.sync.dma_start(out=w1t, in_=w1.ap()[e, d, :, :])
                        for j in range(4):
                            nc.tensor.matmul(ph[:, j * 512:(j + 1) * 512],
                                             xn2T_bf[:, d, :],
                                             w1t[:, j * 512:(j + 1) * 512],
                                             start=(d == 0), stop=False,
                                             skip_group_check=True)
                    b1t = moe.tile([1, H], BF16, tag="brow")
                    nc.sync.dma_start(out=b1t, in_=b1r.ap()[e:e + 1, :])
                    for j in range(4):
                        nc.tensor.matmul(
                            ph[:, j * 512:(j + 1) * 512], ones_row_bf,
                            b1t[0:1, j * 512:(j + 1) * 512],
                            start=False, stop=True, skip_group_check=True)
                    nc.scalar.activation(h1g[:, 0:1024], ph[:, 0:1024], AF.Gelu)
                    nc.scalar.activation(h1g[:, 1024:2048], ph[:, 1024:2048], AF.Gelu)
                    h1gT = moe.tile([HD, 16, BE], BF16, tag="h1gT")
                    for kq in range(4):
                        tpb = pstq.tile([128, 4, BE], BF16, tag="tpq")
                        for q in range(4):
                            kc = kq * 4 + q
                            nc.tensor.transpose(tpb[:, q, :],
                                                h1g[:, kc * 128:(kc + 1) * 128],
                                                ident_bf[0:BE, 0:BE])
                        nc.vector.tensor_copy(h1gT[:, kq * 4:(kq + 1) * 4, :],
                                              tpb[:, :, :])
                    h2g = moe.tile([BE, H], BF16, tag="h2g")
                    ph = psh.tile([BE, H], F32, tag="ph")
                    for kc in range(16):
                        w2t = wstream.tile([HD, H], BF16, tag="wt")
                        nc.sync.dma_start(out=w2t, in_=w2.ap()[e, kc, :, :])
                        for j in range(4):
                            nc.tensor.matmul(ph[:, j * 512:(j + 1) * 512],
                                             h1gT[:, kc, :],
                                             w2t[:, j * 512:(j + 1) * 512],
                                             start=(kc == 0), stop=False,
                                             skip_group_check=True)
                    b2t = moe.tile([1, H], BF16, tag="brow")
                    nc.sync.dma_start(out=b2t, in_=b2r.ap()[e:e + 1, :])
                    for j in range(4):
                        nc.tensor.matmul(
                            ph[:, j * 512:(j + 1) * 512], ones_row_bf,
                            b2t[0:1, j * 512:(j + 1) * 512],
                            start=False, stop=True, skip_group_check=True)
                    nc.scalar.activation(h2g[:, 0:1024], ph[:, 0:1024], AF.Gelu)
                    nc.scalar.activation(h2g[:, 1024:2048], ph[:, 1024:2048], AF.Gelu)
                    # weight rows by the routing weight of this expert, transpose
                    nc.vector.tensor_scalar_mul(h2g, h2g, wloc[:, e:e + 1])
                    h2gT = moe.tile([HD, 16, BE], BF16, tag="h2gT")
                    for kq in range(4):
                        tpb = pstq.tile([128, 4, BE], BF16, tag="tpq")
                        for q in range(4):
                            kc = kq * 4 + q
                            nc.tensor.transpose(tpb[:, q, :],
                                                h2g[:, kc * 128:(kc + 1) * 128],
                                                ident_bf[0:BE, 0:BE])
                        nc.vector.tensor_copy(h2gT[:, kq * 4:(kq + 1) * 4, :],
                                              tpb[:, :, :])
                    for kc in range(16):
                        w3t = wstream.tile([HD, H], BF16, tag="wt")
                        nc.sync.dma_start(out=w3t[:, 0:D], in_=w3.ap()[e, kc, :, :])
                        for j in range(2):
                            nc.tensor.matmul(acc_ps[:, j * 512:(j + 1) * 512],
                                             h2gT[:, kc, :],
                                             w3t[:, j * 512:(j + 1) * 512],
                                             start=(e == 0 and kc == 0), stop=False,
                                             skip_group_check=True)
                    b3t = moe.tile([1, H], BF16, tag="brow")
                    nc.sync.dma_start(out=b3t[0:1, 0:D], in_=b3r.ap()[e:e + 1, :])
                    for j in range(2):
                        nc.tensor.matmul(acc_ps[:, j * 512:(j + 1) * 512],
                                         wrow_bf[0:1, e, :],
                                         b3t[0:1, j * 512:(j + 1) * 512],
                                         start=False, stop=(e == EPC - 1),
                                         skip_group_check=True)
                accp = work.tile([BE, D], F32, tag="accp")
                nc.scalar.copy(accp, acc_ps)
                pstq_ctx.__exit__(None, None, None)
                psacc_ctx.__exit__(None, None, None)
                psh_ctx.__exit__(None, None, None)

            # ===== ReduceScatter #2: each core keeps its 8-token shard =====
            ar2_in = dram.tile([BE, D], F32)
            rs_out = dram.tile([8, D], F32)
            nc.sync.dma_start(out=ar2_in, in_=accp)
            nc.gpsimd.collective_compute(
                "ReduceScatter", OP.add, replica_groups=[list(range(NCORES))],
                ins=[ar2_in.opt()], outs=[rs_out.opt()])
            accs8 = work.tile([8, D], F32, tag="accs")
            nc.sync.dma_start(out=accs8, in_=rs_out)
            with tc.tile_pool(name="psf", bufs=1, space="PSUM") as psf:
                x2p = psf.tile([8, D], F32)
                for j in range(2):
                    nc.tensor.matmul(x2p[:, j * 512:(j + 1) * 512], rsel_sb,
                                     x2_sb[:, j * 512:(j + 1) * 512],
                                     start=True, stop=True)
                ef8 = work.tile([8, D], F32, tag="ef8")
                nc.vector.tensor_add(ef8, x2p[:, :], accs8)
            nc.sync.dma_start(out=ef_out.ap(), in_=ef8)

            # ===== confidence head (sharded) =====
            logit = small.tile([8, 1], F32, tag="logit")
            csc = work.tile([8, D], F32, tag="attns")
            nc.vector.scalar_tensor_tensor(csc, ef8, 1.0, wgb_sb[0:8, :],
                                           op0=OP.mult, op1=OP.mult,
                                           accum_out=logit)
            conf_sb = small.tile([8, 1], F32, tag="conf_sb")
            nc.scalar.activation(conf_sb, logit, AF.Sigmoid, bias=bgb_sb[0:8, :])
            nc.sync.dma_start(out=conf_out.ap(), in_=conf_sb)

            ctx_wstream.__exit__(None, None, None)

    nc.finalize()
    return nc


_NC_CACHE = None


def _get_program():
    global _NC_CACHE
    if _NC_CACHE is None:
        _NC_CACHE = build_program()
    return _NC_CACHE


def prep_inputs(inputs, expert_tokens_top, ln1_s, ln1_b, ln2_s, ln2_b, Wq, Wkv,
                Wproj, bproj, moe_expert_tokens, W1, b1, W2, b2, W3, b3, Wg, bg):
    """Host-side weight preprocessing: slicing per core, LN1 scale fold,
    layout transforms, bf16 casts."""
    f32 = np.float32
    bf16 = ml_dtypes.bfloat16
    x_in = np.ascontiguousarray(np.asarray(inputs, f32).reshape(B * N, D))
    xtop = np.ascontiguousarray(np.asarray(expert_tokens_top, f32))
    ln1_s = np.asarray(ln1_s, f32); ln1_b = np.asarray(ln1_b, f32)
    Wq_f = ln1_s[:, None] * np.asarray(Wq, f32)
    Wkv_f = ln1_s[:, None] * np.asarray(Wkv, f32)
    bq_full = ln1_b @ np.asarray(Wq, f32)
    bkv_full = ln1_b @ np.asarray(Wkv, f32)
    xtb = xtop + np.asarray(bproj, f32)[None, :]
    W1 = np.asarray(W1); W2 = np.asarray(W2); W3 = np.asarray(W3)
    b1 = np.asarray(b1, f32); b2m = np.asarray(b2, f32); b3 = np.asarray(b3, f32)
    moewT = np.ascontiguousarray(np.asarray(moe_expert_tokens, f32).T)  # [D, E]

    common = {
        "x_in": x_in, "xtop": xtop, "xtb": np.ascontiguousarray(xtb),
        "s2row": np.asarray(ln2_s, f32).reshape(1, D),
        "b2row": np.asarray(ln2_b, f32).reshape(1, D),
        "moew": np.ascontiguousarray(moewT.reshape(8, HD, E)),
        "wgrow": np.asarray(Wg, f32).reshape(1, D),
        "bgrow": np.asarray(bg, f32).reshape(1, 1),
    }
    in_maps = []
    for c in range(NCORES):
        h0, h1 = c * HD, (c + 1) * HD
        es = np.zeros((E, EPC), f32)
        for j in range(EPC):
            es[c * EPC + j, j] = 1.0
        m = dict(common)
        m["wk"] = np.ascontiguousarray(Wkv_f[:, h0:h1].reshape(8, HD, HD))
        m["wv"] = np.ascontiguousarray(Wkv_f[:, D + h0:D + h1].reshape(8, HD, HD))
        m["wq"] = np.ascontiguousarray(Wq_f[:, h0:h1].reshape(8, HD, HD))
        m["bk"] = np.ascontiguousarray(bkv_full[h0:h1].reshape(HD, 1))
        m["bv"] = np.ascontiguousarray(bkv_full[D + h0:D + h1].reshape(1, HD))
        m["bq"] = np.ascontiguousarray(bq_full[h0:h1].reshape(HD, 1))
        m["wproj"] = np.ascontiguousarray(np.asarray(Wproj, f32)[h0:h1, :])
        m["esel"] = es
        rs = np.zeros((BE, 8), f32)
        for j in range(8):
            rs[8 * c + j, j] = 1.0
        m["rsel"] = rs
        sl = slice(c * EPC, (c + 1) * EPC)
        m["w1"] = np.ascontiguousarray(W1[sl].reshape(EPC, 8, HD, H)).astype(bf16)
        m["b1r"] = b1[sl].astype(bf16)
        m["w2"] = np.ascontiguousarray(W2[sl].reshape(EPC, 16, HD, H)).astype(bf16)
        m["b2r"] = b2m[sl].astype(bf16)
        m["w3"] = np.ascontiguousarray(W3[sl].reshape(EPC, 16, HD, D)).astype(bf16)
        m["b3r"] = b3[sl].astype(bf16)
        in_maps.append(m)
    return in_maps


def kernel(**inputs):
    nc = _get_program()
    in_maps = prep_inputs(**inputs)
    res = run_bass_kernel_spmd(nc, in_maps, core_ids=list(range(NCORES)))
    r0 = res.results[0]
    ef = np.concatenate([res.results[c]["ef"] for c in range(NCORES)], axis=0)
    conf = np.concatenate([res.results[c]["conf"] for c in range(NCORES)], axis=0)
    ef = ef.reshape(B, E, D).astype(np.float32)
    conf = conf.reshape(B, E, 1).astype(np.float32)
    fused = r0["fused"].astype(np.float32)
    return ef, conf, fused


# revision 40
# speedup vs baseline: 1.0068x; 1.0068x over previous
"""Trainium2 Bass kernel for nn_MoE_Disentangled (moe_routing).

Contract: kernel(**inputs) takes FULL unsharded inputs (as from
reference.setup_inputs()) and returns the FULL outputs
(expert_features [4,16,1024], confidence [4,16,1], fused [4,1024]).

Math: only the first E=16 tokens of the sequence reach the outputs.
Attention mixes tokens (so k/v are needed over all S=1040 tokens), but
only the E expert-token queries matter; everything after attention is
row-wise, so the MoE runs on just B*E=64 tokens.  This is exact.

Sharding over 8 NeuronCores:
  - attention: head-parallel (core c owns head c); k_h/v_h projections over
    all tokens need no communication; o@Wproj row-block partials are
    AllReduce-summed (256 KB).
  - MoE: expert-parallel (core c owns experts 2c,2c+1); weighted
    accumulator partials AllReduce-summed (256 KB).
Attention/routing path is fp32 end-to-end (top-2 routing is sensitive to
score perturbations); expert MLP weights stream as bf16 with fp32 PSUM
accumulation.
"""

import numpy as np
import ml_dtypes

import concourse.bass as bass
import concourse.bacc as bacc
import concourse.tile as tile
import concourse.mybir as mybir
from concourse.bass_utils import run_bass_kernel_spmd
from concourse.masks import make_identity
from concourse.bass import _add_dep_helper

F32 = mybir.dt.float32
BF16 = mybir.dt.bfloat16
F32R = mybir.dt.float32r
AF = mybir.ActivationFunctionType
OP = mybir.AluOpType

B, N, D = 4, 1024, 1024
E, NH = 16, 8
H = 2 * D
HD = D // NH
EPS = 1e-5
NCORES = 8
EPC = E // NCORES          # experts per core
NT = 33                    # token tiles: tile 0 = expert tokens (16 rows), 1..32 = inputs
BE = B * E                 # 64 routed tokens
SCALE = HD ** -0.5


def _bcast_ap(handle, parts, free):
    """AP reading a [1, free] DRAM row broadcast to `parts` partitions."""
    return bass.AP(handle.ap().tensor, 0, [[0, parts], [1, free]])


def build_program():
    nc = bacc.Bacc("TRN2", target_bir_lowering=False, debug=False,
                   num_devices=NCORES)
    def din(name, shape, dtype=F32):
        return nc.dram_tensor(name, list(shape), dtype, kind="ExternalInput")

    x_in = din("x_in", [B * N, D])               # inputs reshaped
    xtop = din("xtop", [E, D])                   # expert_tokens_top
    xtb = din("xtb", [E, D])                     # expert_tokens_top + bproj
    wk = din("wk", [8, HD, HD])                  # ln1_s-folded Wkv k-slice, [d][128,128]
    wv = din("wv", [8, HD, HD])
    wq = din("wq", [8, HD, HD])
    bk = din("bk", [HD, 1])                      # (ln1_b @ Wkv) k-slice
    bq = din("bq", [HD, 1])
    bv = din("bv", [HD, 1])
    wproj = din("wproj", [HD, D])                # Wproj rows of this head
    s2row = din("s2row", [1, D])                 # ln2_s
    b2row = din("b2row", [1, D])                 # ln2_b
    moew = din("moew", [8, HD, E])               # moe_expert_tokens.T chunks
    esel = din("esel", [E, EPC])                 # one-hot picks of this core's experts
    w1 = din("w1", [EPC, 8, HD, H], BF16)
    b1r = din("b1r", [EPC, H], BF16)
    w2 = din("w2", [EPC, 16, HD, H], BF16)
    b2r = din("b2r", [EPC, H], BF16)
    w3 = din("w3", [EPC, 16, HD, D], BF16)
    b3r = din("b3r", [EPC, D], BF16)
    rsel = din("rsel", [BE, 8])                  # one-hot rows of this core's shard
    wgrow = din("wgrow", [1, D])
    bgrow = din("bgrow", [1, 1])

    ef_out = nc.dram_tensor("ef", [8, D], F32, kind="ExternalOutput")
    conf_out = nc.dram_tensor("conf", [8, 1], F32, kind="ExternalOutput")
    fused_out = nc.dram_tensor("fused", [B, D], F32, kind="ExternalOutput")

    with tile.TileContext(nc) as tc:
        with (
            tc.tile_pool(name="consts", bufs=1) as consts,
            tc.tile_pool(name="persist", bufs=1) as persist,
            tc.tile_pool(name="work", bufs=1) as work,
            tc.tile_pool(name="small", bufs=8) as small,
            tc.tile_pool(name="dram", bufs=1, space="DRAM") as dram,
        ):
            # ---- constants ----
            ident = consts.tile([128, 128], F32)
            make_identity(nc, ident)
            ident_bf = consts.tile([64, 64], BF16)
            make_identity(nc, ident_bf)
            ones_row = consts.tile([1, 128], F32)
            nc.vector.memset(ones_row, 1.0)
            ones_row_bf = consts.tile([1, BE], BF16)
            nc.vector.memset(ones_row_bf, 1.0)
            eps_t = consts.tile([128, 1], F32)
            nc.vector.memset(eps_t, EPS)

            wk_st = work.tile([HD, 8, HD], F32, tag="wstage")
            nc.sync.dma_start(out=wk_st, in_=wk.ap().rearrange("d r c -> r d c"))
            wk_sb = consts.tile([HD, 8, HD], F32R)
            nc.vector.tensor_copy(wk_sb, wk_st)
            wv_st = work.tile([HD, 8, HD], F32, tag="wstage")
            nc.sync.dma_start(out=wv_st, in_=wv.ap().rearrange("d r c -> r d c"))
            wv_sb = consts.tile([HD, 8, HD], F32R)
            nc.vector.tensor_copy(wv_sb, wv_st)
            wq_st = work.tile([HD, 8, HD], F32, tag="wstage")
            nc.sync.dma_start(out=wq_st, in_=wq.ap().rearrange("d r c -> r d c"))
            wq_sb = consts.tile([HD, 8, HD], F32R)
            nc.vector.tensor_copy(wq_sb, wq_st)
            moew_sb = consts.tile([HD, 8, E], F32)
            nc.sync.dma_start(out=moew_sb, in_=moew.ap().rearrange("d r c -> r d c"))
            wp_st = work.tile([HD, D], F32, tag="wstage")
            nc.sync.dma_start(out=wp_st, in_=wproj.ap())
            wproj_sb = consts.tile([HD, D], F32R)
            nc.vector.tensor_copy(wproj_sb, wp_st)
            bk_sb = consts.tile([HD, 1], F32)
            nc.sync.dma_start(out=bk_sb, in_=bk.ap())
            bq_sb = consts.tile([HD, 1], F32)
            nc.sync.dma_start(out=bq_sb, in_=bq.ap())
            bv_sb = consts.tile([HD, 1], F32)
            nc.sync.dma_start(out=bv_sb, in_=bv.ap())
            esel_sb = consts.tile([E, EPC], F32)
            nc.sync.dma_start(out=esel_sb, in_=esel.ap())
            rsel_sb = consts.tile([BE, 8], F32)
            nc.sync.dma_start(out=rsel_sb, in_=rsel.ap())

            xtb_sb = consts.tile([BE, D], F32)   # x_top + bproj, broadcast to 64 rows
            nc.sync.dma_start(
                out=xtb_sb, in_=bass.AP(xtb.ap().tensor, 0, [[0, B], [D, E], [1, D]]))
            s2b_sb = consts.tile([BE, D], F32)
            nc.sync.dma_start(out=s2b_sb, in_=_bcast_ap(s2row, BE, D))
            b2b_sb = consts.tile([BE, D], F32)
            nc.sync.dma_start(out=b2b_sb, in_=_bcast_ap(b2row, BE, D))
            wgb_sb = consts.tile([BE, D], F32)
            nc.sync.dma_start(out=wgb_sb, in_=_bcast_ap(wgrow, BE, D))
            bgb_sb = consts.tile([BE, 1], F32)
            nc.sync.dma_start(out=bgb_sb, in_=_bcast_ap(bgrow, BE, 1))

            # ---- persistent results ----
            x2_sb = persist.tile([BE, D], F32)           # post-attention residual

            x_in_r = x_in.ap().rearrange("(b n) d -> b n d", n=N)

            # MoE weight stream pool opened early: lets the scheduler prefetch
            # expert weights during phase 1 and the AllReduce gap.
            wstream = ctx_wstream = tc.tile_pool(name="wstream", bufs=10)
            wstream = wstream.__enter__()

            # ===== phase 1: LN1 + per-head projections over all tokens =====
            kvpool = tc.tile_pool(name="kvpool", bufs=1)
            kvp = kvpool.__enter__()
            k_all = kvp.tile([HD, NT, 128], F32)      # k feature-major per tile
            v_all = kvp.tile([128, NT, HD + 1], F32)  # v token-major (+ones col)
            qT_sb = kvp.tile([HD, E], F32)            # q^T for the 16 expert tokens
            G = 2                                     # input tiles per group
            with (
                tc.tile_pool(name="xp", bufs=4) as xp,
                tc.tile_pool(name="xnp", bufs=3) as xnp,
                tc.tile_pool(name="xntp", bufs=3) as xntp,
                tc.tile_pool(name="vfm", bufs=2) as vfmp,
                tc.tile_pool(name="pst", bufs=4, space="PSUM") as pst,
                tc.tile_pool(name="pskv", bufs=4, space="PSUM") as pskv,
            ):
                nonlocal_sqrt = [None]

                def ln_tile(x_t, P_t, xnT, col):
                    """LN stats + normalize + 8 transposes into xnT[:, :, col:col+P_t]."""
                    stats = small.tile([128, 2, 6], F32, tag="stats")
                    nc.vector.bn_stats(stats[0:P_t, 0, :], x_t[0:P_t, 0:512])
                    nc.vector.bn_stats(stats[0:P_t, 1, :], x_t[0:P_t, 512:1024])
                    mv = small.tile([128, 2], F32, tag="mv")
                    nc.vector.bn_aggr(mv[0:P_t, :], stats[0:P_t, :, :])
                    r_t = small.tile([128, 1], F32, tag="r")
                    nonlocal_sqrt[0] = nc.scalar.activation(
                        r_t[0:P_t, :], mv[0:P_t, 1:2], AF.Sqrt,
                        bias=eps_t[0:P_t, :])
                    nc.vector.reciprocal(r_t[0:P_t, :], r_t[0:P_t, :])
                    nmr = small.tile([128, 1], F32, tag="nmr")
                    nc.vector.tensor_scalar(nmr[0:P_t, :], mv[0:P_t, 0:1],
                                            r_t[0:P_t, :], -1.0,
                                            op0=OP.mult, op1=OP.mult)
                    xn = xnp.tile([128, D], F32, tag="xn")
                    nc.scalar.activation(xn[0:P_t, 0:512], x_t[0:P_t, 0:512],
                                         AF.Identity,
                                         bias=nmr[0:P_t, :], scale=r_t[0:P_t, :])
                    nc.vector.tensor_scalar(xn[0:P_t, 512:1024], x_t[0:P_t, 512:1024],
                                            r_t[0:P_t, :], nmr[0:P_t, :],
                                            op0=OP.mult, op1=OP.add)
                    for dq in range(2):
                        tp = pst.tile([128, 4, 128], F32, tag="tp")
                        for q in range(4):
                            d = dq * 4 + q
                            nc.tensor.transpose(tp[:, q, 0:P_t],
                                                xn[0:P_t, d * 128:(d + 1) * 128],
                                                ident[0:P_t, 0:P_t])
                        if dq == 0:
                            nc.scalar.copy(xnT[:, 0:4, col:col + P_t],
                                           tp[:, :, 0:P_t])
                        else:
                            nc.vector.tensor_copy(xnT[:, 4:8, col:col + P_t],
                                                  tp[:, :, 0:P_t])

                # --- expert-token tile (16 rows, t=0) ---
                x_t = xp.tile([128, D], F32, tag="x_t")
                nc.sync.dma_start(out=x_t[0:E, 0:512], in_=xtop.ap()[:, 0:512])
                nc.sync.dma_start(out=x_t[0:E, 512:1024], in_=xtop.ap()[:, 512:1024])
                xnT0 = xntp.tile([HD, 8, G * 128], F32R, tag="xnT")
                ln_tile(x_t, E, xnT0, 0)
                kp = pskv.tile([128, G * 128], F32, tag="kvq")
                for d in range(8):
                    nc.tensor.matmul(kp[:, 0:E], wk_sb[:, d, :], xnT0[:, d, 0:E],
                                     start=(d == 0), stop=(d == 7))
                nc.scalar.activation(k_all[:, 0, 0:E], kp[:, 0:E], AF.Identity,
                                     bias=bk_sb)
                vp = pskv.tile([128, G * 128], F32, tag="kvq")
                for d in range(8):
                    nc.tensor.matmul(vp[:, 0:E], wv_sb[:, d, :], xnT0[:, d, 0:E],
                                     start=(d == 0), stop=(d == 7))
                vfm = vfmp.tile([HD, G * 128], F32, tag="vfm")
                nc.scalar.activation(vfm[:, 0:E], vp[:, 0:E], AF.Identity,
                                     bias=bv_sb)
                tp = pst.tile([128, 4, 128], F32, tag="tp")
                nc.tensor.transpose(tp[0:E, 0, :], vfm[:, 0:E], ident)
                nc.vector.tensor_copy(v_all[0:E, 0, 0:HD], tp[0:E, 0, :])
                nc.vector.memset(v_all[0:E, 0, HD:HD + 1], 1.0)
                qp = pskv.tile([128, G * 128], F32, tag="kvq")
                for d in range(8):
                    nc.tensor.matmul(qp[:, 0:E], wq_sb[:, d, :], xnT0[:, d, 0:E],
                                     start=(d == 0), stop=(d == 7))
                nc.scalar.activation(qT_sb[:, :], qp[:, 0:E], AF.Identity,
                                     bias=bq_sb)

                # --- input tiles, G at a time (grouped stationary loads) ---
                for g in range(32 // G):
                    t0 = 1 + g * G
                    xnT = xntp.tile([HD, 8, G * 128], F32R, tag="xnT")
                    for i in range(G):
                        x_t = xp.tile([128, D], F32, tag="x_t")
                        rows = x_in.ap()[(t0 - 1 + i) * 128:(t0 + i) * 128, :]
                        nc.sync.dma_start(out=x_t[:, 0:512], in_=rows[:, 0:512])
                        nc.sync.dma_start(out=x_t[:, 512:1024], in_=rows[:, 512:1024])
                        ln_tile(x_t, 128, xnT, i * 128)
                    # k for the whole group (feature-major, contiguous tiles)
                    kp = pskv.tile([128, G * 128], F32, tag="kvq")
                    for d in range(8):
                        nc.tensor.matmul(kp[:, :], wk_sb[:, d, :], xnT[:, d, :],
                                         start=(d == 0), stop=(d == 7))
                    nc.scalar.activation(k_all[:, t0:t0 + G, :], kp[:, :],
                                         AF.Identity, bias=bk_sb)
                    # v feature-major for the group, then transpose per tile
                    vp = pskv.tile([128, G * 128], F32, tag="kvq")
                    for d in range(8):
                        nc.tensor.matmul(vp[:, :], wv_sb[:, d, :], xnT[:, d, :],
                                         start=(d == 0), stop=(d == 7))
                    vfm = vfmp.tile([HD, G * 128], F32, tag="vfm")
                    nc.scalar.activation(vfm[:, :], vp[:, :], AF.Identity,
                                         bias=bv_sb)
                    tp = pst.tile([128, 4, 128], F32, tag="tp")
                    for i in range(G):
                        nc.tensor.transpose(tp[:, i, :], vfm[:, i * 128:(i + 1) * 128],
                                            ident)
                    nc.vector.tensor_copy(v_all[:, t0:t0 + G, 0:HD], tp[:, 0:G, :])
                    nc.vector.memset(v_all[:, t0:t0 + G, HD:HD + 1], 1.0)

            # ===== attention for this head (16 queries per batch) =====
            last_sqrt = nonlocal_sqrt[0]
            ar1_in = dram.tile([BE, D], F32)
            ar1_out = dram.tile([BE, D], F32)
            with (
                tc.tile_pool(name="attn", bufs=4) as attn,
                tc.tile_pool(name="pst", bufs=2, space="PSUM") as pst,
                tc.tile_pool(name="psa", bufs=2, space="PSUM") as psa,
                tc.tile_pool(name="psop", bufs=1, space="PSUM") as psop,
            ):
                for b in range(B):
                    tiles_b = [0] + list(range(8 * b + 1, 8 * b + 9))
                    po = psa.tile([E, HD + 1], F32, tag="po")
                    for i, t in enumerate(tiles_b):
                        P_t = E if t == 0 else 128
                        ps_s = psa.tile([128, E], F32, tag="ps_s")
                        nc.tensor.matmul(ps_s[0:P_t, :], k_all[:, t, 0:P_t], qT_sb,
                                         start=True, stop=True)
                        et = attn.tile([128, E], F32, tag="et")
                        exp_i = nc.scalar.activation(et[0:P_t, :], ps_s[0:P_t, :],
                                                     AF.Exp, scale=SCALE)
                        _add_dep_helper(exp_i.ins, last_sqrt.ins, sync=True,
                                        reason="ACT table: exp after all LN1 sqrt")
                        nc.tensor.matmul(po[:, :], et[0:P_t, :], v_all[0:P_t, t, :],
                                         start=(i == 0), stop=(i == 8),
                                         skip_group_check=True)
                    rden = attn.tile([E, 1], F32, tag="rden")
                    nc.vector.reciprocal(rden, po[:, HD:HD + 1])
                    o_sb = attn.tile([E, HD], F32, tag="o_sb")
                    nc.scalar.activation(o_sb, po[:, 0:HD], AF.Copy, scale=rden)
                    otp = pst.tile([128, 128], F32, tag="tp")
                    nc.tensor.transpose(otp[:, 0:E], o_sb, ident[0:E, 0:E])
                    oT = attn.tile([HD, E], F32R, tag="oT")
                    nc.vector.tensor_copy(oT, otp[:, 0:E])
                    pop = psop.tile([E, D], F32, tag="pop")
                    nc.tensor.matmul(pop[:, 0:512], oT, wproj_sb[:, 0:512],
                                     start=True, stop=True)
                    nc.tensor.matmul(pop[:, 512:1024], oT, wproj_sb[:, 512:1024],
                                     start=True, stop=True)
                    attnp_b = attn.tile([E, D], F32, tag="attnp_b")
                    nc.scalar.copy(attnp_b, pop[:, :])
                    nc.sync.dma_start(out=ar1_in[E * b:E * (b + 1), :], in_=attnp_b)

            # ===== fused passthrough (fills the AllReduce gap) =====
            f0 = work.tile([B, D], F32, tag="f0")
            nc.sync.dma_start(out=f0, in_=x_in_r[:, 0, :])
            f1 = work.tile([B, D], F32, tag="attns")
            nc.sync.dma_start(out=f1, in_=x_in_r[:, 1, :])
            nc.vector.tensor_add(f0, f0, f1)
            nc.vector.tensor_scalar_mul(f0, f0, 0.5)
            nc.sync.dma_start(out=fused_out.ap(), in_=f0)

            # ===== AllReduce #1: sum head partials =====
            kvpool.__exit__(None, None, None)
            nc.gpsimd.collective_compute(
                "AllReduce", OP.add, replica_groups=[list(range(NCORES))],
                ins=[ar1_in.opt()], outs=[ar1_out.opt()])
            attns = work.tile([BE, D], F32, tag="attns")
            nc.sync.dma_start(out=attns, in_=ar1_out)
            nc.vector.tensor_add(x2_sb, attns, xtb_sb)

            # ===== LN2 + routing =====
            with (
                tc.tile_pool(name="moe", bufs=1) as moe,
            ):
                pst1_ctx = tc.tile_pool(name="pst", bufs=2, space="PSUM")
                pst = pst1_ctx.__enter__()
                psr_ctx = tc.tile_pool(name="psr", bufs=2, space="PSUM")
                psr = psr_ctx.__enter__()
                stats2 = small.tile([BE, 2, 6], F32, tag="stats")
                nc.vector.bn_stats(stats2[0:BE, 0, :], x2_sb[:, 0:512])
                nc.vector.bn_stats(stats2[0:BE, 1, :], x2_sb[:, 512:1024])
                mv2 = small.tile([BE, 2], F32, tag="mv")
                nc.vector.bn_aggr(mv2[0:BE], stats2[0:BE])
                r2 = small.tile([BE, 1], F32, tag="r")
                nc.scalar.activation(r2[0:BE], mv2[0:BE, 1:2], AF.Sqrt,
                                     bias=eps_t[0:BE, :])
                nc.vector.reciprocal(r2[0:BE], r2[0:BE])
                nmr2 = small.tile([BE, 1], F32, tag="nmr")
                nc.vector.tensor_scalar(nmr2[0:BE], mv2[0:BE, 0:1], r2[0:BE], -1.0,
                                        op0=OP.mult, op1=OP.mult)
                xn2 = work.tile([BE, D], F32, tag="xn2")
                nc.scalar.activation(xn2, x2_sb, AF.Identity, bias=nmr2[0:BE],
                                     scale=r2[0:BE])
                nc.vector.scalar_tensor_tensor(xn2, xn2, 1.0, s2b_sb,
                                               op0=OP.mult, op1=OP.mult)
                nc.vector.tensor_add(xn2, xn2, b2b_sb)

                xn2T = work.tile([HD, 8, BE], F32, tag="xn2T")
                xn2T_bf = work.tile([HD, 8, BE], BF16, tag="xn2T_bf")
                for dq in range(2):
                    tp = pst.tile([128, 4, BE], F32, tag="tp")
                    for q in range(4):
                        d = dq * 4 + q
                        nc.tensor.transpose(tp[:, q, :],
                                            xn2[:, d * 128:(d + 1) * 128],
                                            ident[0:BE, 0:BE])
                    nc.scalar.copy(xn2T[:, dq * 4:(dq + 1) * 4, :], tp[:, :, :])
                    nc.vector.tensor_copy(xn2T_bf[:, dq * 4:(dq + 1) * 4, :],
                                          tp[:, :, :])

                ps_sc = psr.tile([BE, 128], F32, tag="psr")
                for d in range(8):
                    nc.tensor.matmul(ps_sc[:, 0:E], xn2T[:, d, :], moew_sb[:, d, :],
                                     start=(d == 0), stop=(d == 7))
                sc_sb = small.tile([BE, E], F32, tag="sc_sb")
                nc.vector.tensor_copy(sc_sb, ps_sc[:, 0:E])
                m8 = small.tile([BE, 8], F32, tag="m8")
                nc.vector.max(m8, sc_sb)
                eq1 = small.tile([BE, E], F32, tag="eq1")
                nc.vector.tensor_single_scalar(eq1, sc_sb, m8[:, 0:1], op=OP.is_equal)
                eq2 = small.tile([BE, E], F32, tag="eq2")
                nc.vector.tensor_single_scalar(eq2, sc_sb, m8[:, 1:2], op=OP.is_equal)
                w_sb = small.tile([BE, E], F32, tag="w_sb")
                nc.vector.tensor_add(w_sb, eq1, eq2)
                nc.vector.tensor_scalar_mul(w_sb, w_sb, 0.5)
                # transpose w, select this core's expert columns
                wtp = pst.tile([128, 128], F32, tag="tp")
                nc.tensor.transpose(wtp[0:E, 0:BE], w_sb, ident[0:BE, 0:BE])
                wT = small.tile([E, BE], F32, tag="wT")
                nc.vector.tensor_copy(wT, wtp[0:E, 0:BE])
                wlp = psr.tile([BE, 128], F32, tag="psr")
                nc.tensor.matmul(wlp[:, 0:EPC], wT, esel_sb, start=True, stop=True)
                wloc = small.tile([BE, EPC], F32, tag="wloc")
                nc.vector.tensor_copy(wloc, wlp[:, 0:EPC])
                wrow_bf = small.tile([1, EPC, BE], BF16, tag="wrow_bf")
                for e in range(EPC):
                    wrp = psr.tile([BE, 128], F32, tag="psr")
                    nc.tensor.matmul(wrp[0:1, 0:BE], esel_sb[:, e:e + 1], wT,
                                     start=True, stop=True)
                    nc.vector.tensor_copy(wrow_bf[0:1, e, :], wrp[0:1, 0:BE])

                psr_ctx.__exit__(None, None, None)
                pst1_ctx.__exit__(None, None, None)
                # ===== expert MLPs (bf16), weighted-accumulated in PSUM =====
                psh_ctx = tc.tile_pool(name="psh", bufs=1, space="PSUM")
                psh = psh_ctx.__enter__()
                psacc_ctx = tc.tile_pool(name="psacc", bufs=1, space="PSUM")
                psacc = psacc_ctx.__enter__()
                pstq_ctx = tc.tile_pool(name="pstq", bufs=2, space="PSUM")
                pstq = pstq_ctx.__enter__()
                acc_ps = psacc.tile([BE, D], F32)
                for e in range(EPC):
                    h1g = moe.tile([BE, H], BF16, tag="h1g")
                    ph = psh.tile([BE, H], F32, tag="ph")
                    for d in range(8):
                        w1t = wstream.tile([HD, H], BF16, tag="wt")
                        nc.sync.dma_start(out=w1t, in_=w1.ap()[e, d, :, :])
                        for j in range(4):
                            nc.tensor.matmul(ph[:, j * 512:(j + 1) * 512],
                                             xn2T_bf[:, d, :],
                                             w1t[:, j * 512:(j + 1) * 512],
                                             start=(d == 0), stop=False,
                                             skip_group_check=True)
                    b1t = moe.tile([1, H], BF16, tag="brow")
                    nc.sync.dma_start(out=b1t, in_=b1r.ap()[e:e + 1, :])
                    for j in range(4):
                        nc.tensor.matmul(
                            ph[:, j * 512:(j + 1) * 512], ones_row_bf,
                            b1t[0:1, j * 512:(j + 1) * 512],
                            start=False, stop=True, skip_group_check=True)
                    nc.scalar.activation(h1g[:, 0:1024], ph[:, 0:1024], AF.Gelu)
                    nc.scalar.activation(h1g[:, 1024:2048], ph[:, 1024:2048], AF.Gelu)
                    h1gT = moe.tile([HD, 16, BE], BF16, tag="h1gT")
                    for kq in range(4):
                        tpb = pstq.tile([128, 4, BE], BF16, tag="tpq")
                        for q in range(4):
                            kc = kq * 4 + q
                            nc.tensor.transpose(tpb[:, q, :],
                                                h1g[:, kc * 128:(kc + 1) * 128],
                                                ident_bf[0:BE, 0:BE])
                        nc.vector.tensor_copy(h1gT[:, kq * 4:(kq + 1) * 4, :],
                                              tpb[:, :, :])
                    h2g = moe.tile([BE, H], BF16, tag="h2g")
                    ph = psh.tile([BE, H], F32, tag="ph")
                    for kc in range(16):
                        w2t = wstream.tile([HD, H], BF16, tag="wt")
                        nc.sync.dma_start(out=w2t, in_=w2.ap()[e, kc, :, :])
                        for j in range(4):
                            nc.tensor.matmul(ph[:, j * 512:(j + 1) * 512],
                                             h1gT[:, kc, :],
                                             w2t[:, j * 512:(j + 1) * 512],
                                             start=(kc == 0), stop=False,
                                             skip_group_check=True)
                    b2t = moe.tile([1, H], BF16, tag="brow")
                    nc.sync.dma_start(out=b2t, in_=b2r.ap()[e:e + 1, :])
                    for j in range(4):
                        nc.tensor.matmul(
                            ph[:, j * 512:(j + 1) * 512], ones_row_bf,
                            b2t[0:1, j * 512:(j + 1) * 512],
                            start=False, stop=True, skip_group_check=True)
                    nc.scalar.activation(h2g[:, 0:1024], ph[:, 0:1024], AF.Gelu)
                    nc.scalar.activation(h2g[:, 1024:2048], ph[:, 1024:2048], AF.Gelu)
                    # weight rows by the routing weight of this expert, transpose
                    nc.vector.tensor_scalar_mul(h2g, h2g, wloc[:, e:e + 1])
                    h2gT = moe.tile([HD, 16, BE], BF16, tag="h2gT")
                    for kq in range(4):
                        tpb = pstq.tile([128, 4, BE], BF16, tag="tpq")
                        for q in range(4):
                            kc = kq * 4 + q
                            nc.tensor.transpose(tpb[:, q, :],
                                                h2g[:, kc * 128:(kc + 1) * 128],
                                                ident_bf[0:BE, 0:BE])
                        nc.vector.tensor_copy(h2gT[:, kq * 4:(kq + 1) * 4, :],
                                              tpb[:, :, :])
                    for kc in range(16):
                        w3t = wstream.tile([HD, H], BF16, tag="wt")
                        nc.sync.dma_start(out=w3t[:, 0:D], in_=w3.ap()[e, kc, :, :])
                        for j in range(2):
                            nc.tensor.matmul(acc_ps[:, j * 512:(j + 1) * 512],
                                             h2gT[:, kc, :],
                                             w3t[:, j * 512:(j + 1) * 512],
                                             start=(e == 0 and kc == 0), stop=False,
                                             skip_group_check=True)
                    b3t = moe.tile([1, H], BF16, tag="brow")
                    nc.sync.dma_start(out=b3t[0:1, 0:D], in_=b3r.ap()[e:e + 1, :])
                    for j in range(2):
                        nc.tensor.matmul(acc_ps[:, j * 512:(j + 1) * 512],
                                         wrow_bf[0:1, e, :],
                                         b3t[0:1, j * 512:(j + 1) * 512],
                                         start=False, stop=(e == EPC - 1),
                                         skip_group_check=True)
                accp = work.tile([BE, D], F32, tag="accp")
                nc.scalar.copy(accp, acc_ps)
                pstq_ctx.__exit__(None, None, None)
                psacc_ctx.__exit__(None, None, None)
                psh_ctx.__exit__(None, None, None)

            # ===== ReduceScatter #2: each core keeps its 8-token shard =====
            ar2_in = dram.tile([BE, D], F32)
            rs_out = dram.tile([8, D], F32)
            nc.sync.dma_start(out=ar2_in, in_=accp)
            nc.gpsimd.collective_compute(
                "ReduceScatter", OP.add, replica_groups=[list(range(NCORES))],
                ins=[ar2_in.opt()], outs=[rs_out.opt()])
            accs8 = work.tile([8, D], F32, tag="accs")
            nc.sync.dma_start(out=accs8, in_=rs_out)
            with tc.tile_pool(name="psf", bufs=1, space="PSUM") as psf:
                x2p = psf.tile([8, D], F32)
                for j in range(2):
                    nc.tensor.matmul(x2p[:, j * 512:(j + 1) * 512], rsel_sb,
                                     x2_sb[:, j * 512:(j + 1) * 512],
                                     start=True, stop=True)
                ef8 = work.tile([8, D], F32, tag="ef8")
                nc.vector.tensor_add(ef8, x2p[:, :], accs8)
            nc.sync.dma_start(out=ef_out.ap(), in_=ef8)

            # ===== confidence head (sharded) =====
            logit = small.tile([8, 1], F32, tag="logit")
            csc = work.tile([8, D], F32, tag="attns")
            nc.vector.scalar_tensor_tensor(csc, ef8, 1.0, wgb_sb[0:8, :],
                                           op0=OP.mult, op1=OP.mult,
                                           accum_out=logit)
            conf_sb = small.tile([8, 1], F32, tag="conf_sb")
            nc.scalar.activation(conf_sb, logit, AF.Sigmoid, bias=bgb_sb[0:8, :])
            nc.sync.dma_start(out=conf_out.ap(), in_=conf_sb)

            ctx_wstream.__exit__(None, None, None)

    nc.finalize()
    return nc


_NC_CACHE = None


def _get_program():
    global _NC_CACHE
    if _NC_CACHE is None:
        _NC_CACHE = build_program()
    return _NC_CACHE


def prep_inputs(inputs, expert_tokens_top, ln1_s, ln1_b, ln2_s, ln2_b, Wq, Wkv,
                Wproj, bproj, moe_expert_tokens, W1, b1, W2, b2, W3, b3, Wg, bg):
    """Host-side weight preprocessing: slicing per core, LN1 scale fold,
    layout transforms, bf16 casts."""
    f32 = np.float32
    bf16 = ml_dtypes.bfloat16
    x_in = np.ascontiguousarray(np.asarray(inputs, f32).reshape(B * N, D))
    xtop = np.ascontiguousarray(np.asarray(expert_tokens_top, f32))
    ln1_s = np.asarray(ln1_s, f32); ln1_b = np.asarray(ln1_b, f32)
    Wq_f = ln1_s[:, None] * np.asarray(Wq, f32)
    Wkv_f = ln1_s[:, None] * np.asarray(Wkv, f32)
    bq_full = ln1_b @ np.asarray(Wq, f32)
    bkv_full = ln1_b @ np.asarray(Wkv, f32)
    xtb = xtop + np.asarray(bproj, f32)[None, :]
    W1 = np.asarray(W1); W2 = np.asarray(W2); W3 = np.asarray(W3)
    b1 = np.asarray(b1, f32); b2m = np.asarray(b2, f32); b3 = np.asarray(b3, f32)
    moewT = np.ascontiguousarray(np.asarray(moe_expert_tokens, f32).T)  # [D, E]

    common = {
        "x_in": x_in, "xtop": xtop, "xtb": np.ascontiguousarray(xtb),
        "s2row": np.asarray(ln2_s, f32).reshape(1, D),
        "b2row": np.asarray(ln2_b, f32).reshape(1, D),
        "moew": np.ascontiguousarray(moewT.reshape(8, HD, E)),
        "wgrow": np.asarray(Wg, f32).reshape(1, D),
        "bgrow": np.asarray(bg, f32).reshape(1, 1),
    }
    in_maps = []
    for c in range(NCORES):
        h0, h1 = c * HD, (c + 1) * HD
        es = np.zeros((E, EPC), f32)
        for j in range(EPC):
            es[c * EPC + j, j] = 1.0
        m = dict(common)
        m["wk"] = np.ascontiguousarray(Wkv_f[:, h0:h1].reshape(8, HD, HD))
        m["wv"] = np.ascontiguousarray(Wkv_f[:, D + h0:D + h1].reshape(8, HD, HD))
        m["wq"] = np.ascontiguousarray(Wq_f[:, h0:h1].reshape(8, HD, HD))
        m["bk"] = np.ascontiguousarray(bkv_full[h0:h1].reshape(HD, 1))
        m["bv"] = np.ascontiguousarray(bkv_full[D + h0:D + h1].reshape(1, HD))
        m["bq"] = np.ascontiguousarray(bq_full[h0:h1].reshape(HD, 1))
        m["wproj"] = np.ascontiguousarray(np.asarray(Wproj, f32)[h0:h1, :])
        m["esel"] = es
        rs = np.zeros((BE, 8), f32)
        for j in range(8):
            rs[8 * c + j, j] = 1.0
        m["rsel"] = rs
        sl = slice(c * EPC, (c + 1) * EPC)
        m["w1"] = np.ascontiguousarray(W1[sl].reshape(EPC, 8, HD, H)).astype(bf16)
        m["b1r"] = b1[sl].astype(bf16)
        m["w2"] = np.ascontiguousarray(W2[sl].reshape(EPC, 16, HD, H)).astype(bf16)
        m["b2r"] = b2m[sl].astype(bf16)
        m["w3"] = np.ascontiguousarray(W3[sl].reshape(EPC, 16, HD, D)).astype(bf16)
        m["b3r"] = b3[sl].astype(bf16)
        in_maps.append(m)
    return in_maps


def kernel(**inputs):
    nc = _get_program()
    in_maps = prep_inputs(**inputs)
    res = run_bass_kernel_spmd(nc, in_maps, core_ids=list(range(NCORES)))
    r0 = res.results[0]
    ef = np.concatenate([res.results[c]["ef"] for c in range(NCORES)], axis=0)
    conf = np.concatenate([res.results[c]["conf"] for c in range(NCORES)], axis=0)
    ef = ef.reshape(B, E, D).astype(np.float32)
    conf = conf.reshape(B, E, 1).astype(np.float32)
    fused = r0["fused"].astype(np.float32)
    return ef, conf, fused
